# revision 8
# baseline (speedup 1.0000x reference)
"""Trainium2 Bass kernel for nn_DDNWithResidualLoss.

Contract: kernel(**inputs) takes the FULL unsharded inputs (numpy arrays,
keyed as in reference.setup_inputs()) and returns the FULL output (the two
scalar losses). The batch dim B=8 is sharded 1 image per NeuronCore across
8 cores; per-core partial weighted sums are combined on the host (the
cross-device psum is 16 floats).

Key observation: the per-pixel target bin t takes at most 17 distinct
values per image (16 boxes + background), so the channel gather
x[t[p], p] is a one-hot matmul: out[p, j] = sum_c x[c, p] * H[c, j] with
H the [81, 17] one-hot of the image's candidate bins, followed by a
17-way select keyed on t. The same matmul (with a ones column prepended)
produces the softmax denominator sum_c exp(x[c, p]).

Device kernel (per core, one image, natural [81, 30720] layout):
  - stream logits/residual blocks in; ScalarEngine computes e = exp(x).
  - per 128-pixel chunk, two PE matmuls: [ones|H]^T-gather of e (gives
    denominator s and the 17 exp-candidates) and H^T-gather of r.
  - 17-way select via tensor_scalar(is_equal) masks + copy_predicated.
  - epilogue on [128, 240] tiles: p = exp(ln e_t - ln s), focal weight,
    log-loss + L1-residual loss, weighted partial sums per partition.
The box rasterization + LID depth binning involve only the tiny box
inputs (640 floats); they are replicated bit-exactly on the host in
float32 and shipped as small per-pixel auxiliary planes.
"""

import numpy as np

# ---------------- problem constants (hardcoded per contract) ----------------
B, D, H, W = 8, 80, 96, 320
C = D + 1              # 81 channels
HW = H * W             # 30720 pixels
P = 128                # SBUF partitions per chunk
NCH = HW // P          # 240 chunks of 128 pixels
NCAND = 17             # max distinct target bins per image (16 boxes + bg)
XJ = NCAND + 1         # ones column + candidates
ALPHA, GAMMA = 0.25, 2.0
FG_W, BG_W = 13.0, 1.0
DEPTH_MIN, DEPTH_MAX = 0.001, 60.0
EPS = 1e-8
N_CORES = 8

f32 = np.float32


# ---------------- host-side reference-exact target computation ----------------
def _host_targets(gt_boxes2d, num_gt_per_img, gt_center_depth):
    """Bit-exact float32 replication of the reference's rasterization+binning.

    Returns per-pixel planes (B, H, W): depth bin target (int32),
    residual target (f32), balancer weight (f32).
    """
    gt_boxes2d = np.asarray(gt_boxes2d, f32)
    gt_center_depth = np.asarray(gt_center_depth, f32)
    num_gt = np.asarray(num_gt_per_img, np.int64)

    u1 = np.floor(gt_boxes2d[:, 0]).astype(np.int32)
    v1 = np.floor(gt_boxes2d[:, 1]).astype(np.int32)
    u2 = np.ceil(gt_boxes2d[:, 2]).astype(np.int32)
    v2 = np.ceil(gt_boxes2d[:, 3]).astype(np.int32)
    ntot = gt_boxes2d.shape[0]

    # jnp.repeat(..., total_repeat_length=ntot): truncate, or pad with the
    # final value (matches jax semantics for the padded tail).
    rep = np.repeat(np.arange(B), np.clip(num_gt, 0, None))
    if len(rep) >= ntot:
        rep = rep[:ntot]
    else:
        pad_val = rep[-1] if len(rep) else 0
        rep = np.concatenate([rep, np.full(ntot - len(rep), pad_val, rep.dtype)])

    dm = np.full((B, H, W), DEPTH_MAX, f32)
    fg = np.zeros((B, H, W), bool)
    for i in range(ntot):
        b = int(rep[i])
        ys = slice(max(int(v1[i]), 0), max(int(v2[i]), 0))
        xs = slice(max(int(u1[i]), 0), max(int(u2[i]), 0))
        dm[b, ys, xs] = np.minimum(dm[b, ys, xs], gt_center_depth[i])
        fg[b, ys, xs] = True

    num_bins = D
    bin_size = f32(2.0 * (DEPTH_MAX - DEPTH_MIN) / (num_bins * (1 + num_bins)))
    with np.errstate(invalid="ignore"):
        idx = f32(-0.5) + f32(0.5) * np.sqrt(
            f32(1.0) + f32(8.0) * (dm - f32(DEPTH_MIN)) / bin_size, dtype=f32
        )
        bad = (idx < 0) | (idx > num_bins) | ~np.isfinite(idx)
        tgt = np.where(bad, num_bins, np.floor(np.where(bad, 0, idx))).astype(np.int32)

    bi = np.arange(num_bins, dtype=f32)
    bin_value = (bi + f32(0.5)) ** 2 * bin_size / f32(2.0) - bin_size / f32(8.0) + f32(DEPTH_MIN)
    bin_values = np.concatenate([bin_value, np.array([DEPTH_MAX], f32)])

    res_tgt = (dm - bin_values[tgt]).astype(f32)
    wgt = np.where(fg, f32(FG_W), f32(BG_W))
    return tgt, res_tgt, wgt


def _pmajor(plane):
    """(H*W,) raster vector -> [128, 240] tile, pixel i=128k+p at [p, k]."""
    return np.ascontiguousarray(plane.reshape(NCH, P).T)


# ---------------- device program ----------------
_PROGRAM = None

BLK = 3840               # pixels per staged DMA block (15 KB/partition)
NBLK = HW // BLK         # 8 blocks
CPB = BLK // P           # 30 chunks per block
GRP = 15                 # chunks per PSUM group (15*18=270, 15*17=255 <= 512)


def _build_program(loop_iters=None):
    """Build the SPMD program. loop_iters (benchmark only): wrap the body in
    an on-device For loop so one NEFF executes the kernel body N times,
    letting wall-clock measurements amortize launch/transfer overhead."""
    import concourse.tile as tile
    from concourse import bacc, mybir
    from contextlib import ExitStack, nullcontext

    dt = mybir.dt
    Alu = mybir.AluOpType
    Act = mybir.ActivationFunctionType

    nc = bacc.Bacc("TRN2", target_bir_lowering=False, debug=False)

    x_d = nc.declare_dram_parameter("x", [C, HW], dt.float32, isOutput=False)
    r_d = nc.declare_dram_parameter("r", [C, HW], dt.float32, isOutput=False)
    rh_d = nc.declare_dram_parameter("rh", [C, XJ], dt.float32, isOutput=False)
    cb_d = nc.declare_dram_parameter("cb", [P, NCAND], dt.float32, isOutput=False)
    t_d = nc.declare_dram_parameter("tf", [P, NCH], dt.float32, isOutput=False)
    rt_d = nc.declare_dram_parameter("rt", [P, NCH], dt.float32, isOutput=False)
    w_d = nc.declare_dram_parameter("w", [P, NCH], dt.float32, isOutput=False)
    out_d = nc.declare_dram_parameter("out", [P, 2], dt.float32, isOutput=True)

    with tile.TileContext(nc) as tc, ExitStack() as ctx:
        const_p = ctx.enter_context(tc.tile_pool(name="const", bufs=1))
        stage_p = ctx.enter_context(tc.tile_pool(name="stage", bufs=2))
        psum_p = ctx.enter_context(tc.tile_pool(name="psum", bufs=2, space="PSUM"))
        small_p = ctx.enter_context(tc.tile_pool(name="small", bufs=1))
        mask_p = ctx.enter_context(tc.tile_pool(name="mask", bufs=2))

        rh = const_p.tile([C, XJ], dt.float32)
        nc.sync.dma_start(out=rh[:], in_=rh_d[:])
        cb = const_p.tile([P, NCAND], dt.float32)
        nc.sync.dma_start(out=cb[:], in_=cb_d[:])
        eps_t = const_p.tile([P, 1], dt.float32)
        nc.gpsimd.memset(eps_t[:], EPS)
        t_t = small_p.tile([P, NCH], dt.float32)
        nc.sync.dma_start(out=t_t[:], in_=t_d[:])
        rt_t = small_p.tile([P, NCH], dt.float32)
        nc.sync.dma_start(out=rt_t[:], in_=rt_d[:])
        w_t = small_p.tile([P, NCH], dt.float32)
        nc.sync.dma_start(out=w_t[:], in_=w_d[:])

        xc = small_p.tile([P, NCH * XJ], dt.float32)   # [s | e-candidates] per chunk
        rc = small_p.tile([P, NCH * NCAND], dt.float32)  # r-candidates per chunk

        loop_cm = (tc.For_i(0, loop_iters, 1, hint_engines=(nc.tensor.engine,))
                   if loop_iters else nullcontext())
        ctx.enter_context(loop_cm)

        for blk in range(NBLK):
            xs = stage_p.tile([C, BLK], dt.float32, tag="xs")
            nc.sync.dma_start(out=xs[:], in_=x_d[:, blk * BLK:(blk + 1) * BLK])
            rs = stage_p.tile([C, BLK], dt.float32, tag="rs")
            nc.sync.dma_start(out=rs[:], in_=r_d[:, blk * BLK:(blk + 1) * BLK])
            es = stage_p.tile([C, BLK], dt.float32, tag="es")
            nc.scalar.activation(es[:], xs[:], Act.Exp)
            for g in range(CPB // GRP):
                xg = psum_p.tile([P, GRP * XJ], dt.float32, tag="xg", space="PSUM")
                rg = psum_p.tile([P, GRP * NCAND], dt.float32, tag="rg", space="PSUM")
                for j in range(GRP):
                    kl = g * GRP + j
                    nc.tensor.matmul(
                        xg[:, j * XJ:(j + 1) * XJ],
                        es[:, kl * P:(kl + 1) * P], rh[:],
                        start=True, stop=True)
                    nc.tensor.matmul(
                        rg[:, j * NCAND:(j + 1) * NCAND],
                        rs[:, kl * P:(kl + 1) * P], rh[:, 1:XJ],
                        start=True, stop=True)
                kg = blk * (CPB // GRP) + g
                nc.vector.tensor_copy(
                    xc[:, kg * GRP * XJ:(kg + 1) * GRP * XJ], xg[:])
                nc.vector.tensor_copy(
                    rc[:, kg * GRP * NCAND:(kg + 1) * GRP * NCAND], rg[:])

        # ---------------- select at target bin ----------------
        xcv = xc[:].rearrange("p (k j) -> p k j", j=XJ)
        rcv = rc[:].rearrange("p (k j) -> p k j", j=NCAND)
        s_t = small_p.tile([P, NCH], dt.float32)
        nc.vector.tensor_copy(s_t[:], xcv[:, :, 0])
        et = small_p.tile([P, NCH], dt.float32)
        nc.vector.tensor_copy(et[:], xcv[:, :, 1])
        rp = small_p.tile([P, NCH], dt.float32)
        nc.vector.tensor_copy(rp[:], rcv[:, :, 0])
        for j in range(1, NCAND):
            mask = mask_p.tile([P, NCH], dt.uint8, tag="mask")
            nc.vector.tensor_scalar(mask[:], t_t[:], cb[:, j:j + 1], None,
                                    op0=Alu.is_equal)
            nc.vector.copy_predicated(et[:], mask[:], xcv[:, :, 1 + j])
            nc.vector.copy_predicated(rp[:], mask[:], rcv[:, :, j])

        # ---------------- epilogue on [128, 240] ----------------
        ln_et = small_p.tile([P, NCH], dt.float32)
        nc.scalar.activation(ln_et[:], et[:], Act.Ln)
        ln_s = small_p.tile([P, NCH], dt.float32)
        nc.scalar.activation(ln_s[:], s_t[:], Act.Ln)
        q = small_p.tile([P, NCH], dt.float32)
        nc.vector.tensor_sub(q[:], ln_et[:], ln_s[:])
        praw = small_p.tile([P, NCH], dt.float32)
        nc.scalar.activation(praw[:], q[:], Act.Exp)          # p = e_t / s
        lnp = small_p.tile([P, NCH], dt.float32)
        nc.scalar.activation(lnp[:], praw[:], Act.Ln, bias=eps_t[:])  # log(p+eps)
        u = small_p.tile([P, NCH], dt.float32)
        nc.vector.tensor_scalar(u[:], praw[:], -1.0, 1.0, op0=Alu.mult, op1=Alu.add)
        focal = small_p.tile([P, NCH], dt.float32)
        nc.scalar.activation(focal[:], u[:], Act.Square, scale=0.5)   # 0.25*(1-p)^2
        m1 = small_p.tile([P, NCH], dt.float32)
        nc.vector.tensor_mul(m1[:], focal[:], lnp[:])
        m1w = small_p.tile([P, NCH], dt.float32)
        nc.vector.tensor_mul(m1w[:], m1[:], w_t[:])
        acc0 = small_p.tile([P, 1], dt.float32)
        nc.vector.tensor_reduce(acc0[:], m1w[:], axis=mybir.AxisListType.X,
                                op=Alu.add)
        part = small_p.tile([P, 2], dt.float32)
        nc.vector.tensor_scalar(part[:, 0:1], acc0[:], -1.0, None, op0=Alu.mult)

        dres = small_p.tile([P, NCH], dt.float32)
        nc.vector.tensor_sub(dres[:], rp[:], rt_t[:])
        ares = small_p.tile([P, NCH], dt.float32)
        nc.scalar.activation(ares[:], dres[:], Act.Abs)
        m2 = small_p.tile([P, NCH], dt.float32)
        nc.vector.tensor_mul(m2[:], ares[:], focal[:])
        m2w = small_p.tile([P, NCH], dt.float32)
        nc.vector.tensor_mul(m2w[:], m2[:], w_t[:])
        nc.vector.tensor_reduce(part[:, 1:2], m2w[:], axis=mybir.AxisListType.X,
                                op=Alu.add)
        nc.sync.dma_start(out=out_d[:], in_=part[:])

    nc.compile()
    return nc


def _get_program():
    global _PROGRAM
    if _PROGRAM is None:
        _PROGRAM = _build_program()
    return _PROGRAM


LAST_RESULTS = None  # populated with the BassKernelResults of the last run


def kernel(depth_logits, depth_residuals, gt_boxes2d, num_gt_per_img, gt_center_depth):
    global LAST_RESULTS
    from concourse.bass_utils import run_bass_kernel_spmd

    depth_logits = np.ascontiguousarray(np.asarray(depth_logits, f32))
    depth_residuals = np.ascontiguousarray(np.asarray(depth_residuals, f32))

    tgt, res_tgt, wgt = _host_targets(gt_boxes2d, num_gt_per_img, gt_center_depth)

    nc = _get_program()
    in_maps = []
    for b in range(N_CORES):
        tgt_b = tgt[b].reshape(HW)
        c_list = np.unique(tgt_b)
        assert len(c_list) <= NCAND, f"more than {NCAND} distinct bins"
        c_list = np.concatenate(
            [c_list, np.full(NCAND - len(c_list), c_list[0], c_list.dtype)])
        rh = np.zeros((C, XJ), f32)
        rh[:, 0] = 1.0
        rh[c_list, np.arange(1, XJ)] = 1.0
        cb = np.tile(c_list.astype(f32), (P, 1))
        in_maps.append({
            "x": depth_logits[b].reshape(C, HW),
            "r": depth_residuals[b].reshape(C, HW),
            "rh": rh,
            "cb": np.ascontiguousarray(cb),
            "tf": _pmajor(tgt_b.astype(f32)),
            "rt": _pmajor(res_tgt[b].reshape(HW)),
            "w": _pmajor(wgt[b].reshape(HW)),
        })

    res = run_bass_kernel_spmd(nc, in_maps, list(range(N_CORES)))
    LAST_RESULTS = res

    acc = np.zeros(2, np.float64)
    for b in range(N_CORES):
        acc += np.asarray(res.results[b]["out"], np.float64).sum(axis=0)
    num_pixels = float(B * H * W)
    map_loss = f32(acc[0] / num_pixels)
    res_loss = f32(acc[1] / num_pixels)
    return map_loss, res_loss
